# revision 21
# baseline (speedup 1.0000x reference)
"""Trainium2 Bass kernel for a top-2 MoE layer (T=2048, H=2048, I=1408, E=8).

Strategy: expert-parallel over 8 NeuronCores. The host dispatches tokens:
for each expert e it gathers up to C=512 of the tokens routed to e; the
handful of overflow (token, expert) pairs beyond 512 on the busiest experts
(66 pairs for the seed-0 routing) are computed exactly on the host in fp32,
so the device does exactly ONE SPMD launch with a balanced 512-column
capacity on every core.

Device kernel (per core), transposed layout so no on-device transposes:
  stage 1: guT[2816, C] = w13 @ xT         (22 x 16 matmuls, K-tiles of 128)
  stage 2: actT[1408, C] = silu(gT) * uT   (ScalarE Silu + VectorE mul)
  stage 3: yT[2048, C] = w2 @ actT         (16 x 11 matmuls)
Matmuls run in fp16 (full PE rate: 78.6 TF/s/core; fp8 DoubleRow would be
2x but its 3-bit mantissa costs ~4e-2 rel err vs the 2e-2 budget).

Perf notes vs the naive schedule (single qSyncDynamicHW queue ~170 GB/s):
  - DMAs are split across BOTH hardware DGE queues (sync + scalar
    engines) to approach the ~358 GB/s per-core HBM ceiling, with the
    issue order hand-arranged so stage-1 never starves after startup.
  - All w2 tiles are prefetched during stage 1 (they used to arrive
    ~10us late, stalling the tail of stage 3).
  - A burst of dummy warmup matmuls runs during the initial x/w DMA
    window so the PE clock is fully ramped (2.4 GHz p-state) when the
    first real matmul issues.
  - y is written back in fp16 (half the writeback bytes; adds ~2e-4
    rel err against a 2e-2 budget).
"""

import sys

if "/opt/trn_rl_repo" not in sys.path:
    sys.path.insert(0, "/opt/trn_rl_repo")

import os
import numpy as np
from contextlib import ExitStack

import concourse.bass as bass
import concourse.tile as tile
from concourse import bacc, mybir

T, H, I, E, K = 2048, 2048, 1408, 8, 2
C = 512                      # token capacity per expert (one PSUM bank)
HT = H // 128                # 16 K-tiles over H
IT = I // 128                # 11 K-tiles over I
BT = 2 * I // 128            # 22 row-blocks of guT

DT = mybir.dt.float16
NP_DT = np.float16

_cache: dict = {}


def _build_nc():
    """Build + compile the per-core FFN program (same program on all cores)."""
    nc = bacc.Bacc("TRN2", target_bir_lowering=False, debug=False, num_devices=E)
    # x packed partition-major: x_d[p, k*C + c] = x[token c, h = k*128 + p]
    x_d = nc.dram_tensor("x_sb", [128, HT * C], DT, kind="ExternalInput")
    w13_d = nc.dram_tensor("w13_sb", [BT, 128, HT * 128], DT, kind="ExternalInput")
    w2_d = nc.dram_tensor("w2_sb", [HT, 128, IT * 128], DT, kind="ExternalInput")
    y_d = nc.dram_tensor("y_sb", [HT, 128, C], DT, kind="ExternalOutput")

    AF = mybir.ActivationFunctionType
    F32 = mybir.dt.float32

    with tile.TileContext(nc) as tc, ExitStack() as ctx:
        # every weight tile gets its own buffer: load DMA issues then have no
        # WAR deps, so both DGE queues fill their full backlog at t=0 and the
        # issuing engines never block mid-stream (a blocked dma_start on the
        # scalar engine deadlocks against the silu -> PSUM-ring WAR chain)
        xp = ctx.enter_context(tc.tile_pool(name="x", bufs=1))
        wpg = ctx.enter_context(tc.tile_pool(name="wg", bufs=IT))
        wpu = ctx.enter_context(tc.tile_pool(name="wu", bufs=IT))
        w2p = ctx.enter_context(tc.tile_pool(name="w2", bufs=16))
        ap_ = ctx.enter_context(tc.tile_pool(name="act", bufs=1))
        sp = ctx.enter_context(tc.tile_pool(name="tmp", bufs=2))
        psg = ctx.enter_context(
            tc.tile_pool(name="psg", bufs=4, space=bass.MemorySpace.PSUM)
        )
        psy = ctx.enter_context(
            tc.tile_pool(name="psy", bufs=4, space=bass.MemorySpace.PSUM)
        )

        # (no PE warmup: N=128 dummy matmuls run at ~50% utilization — the
        # 128-cycle stationary load isn't hidden — so the clock never ramps
        # and they only delay the first real matmul; the early real matmuls
        # are DMA-paced anyway, which hides the p-state ramp)

        # --- tiles -----------------------------------------------------
        # x as 8 pair-tiles [128, 2C] (2 KB contiguous per partition per DMA)
        xt = [xp.tile([128, 2 * C], DT, tag=f"xp{j}", name=f"xp{j}") for j in range(HT // 2)]
        wg_t = [wpg.tile([128, HT * 128], DT, tag="wg", name=f"wg{m}") for m in range(IT)]
        wu_t = [wpu.tile([128, HT * 128], DT, tag="wu", name=f"wu{m}") for m in range(IT)]
        w2_t = [w2p.tile([128, IT * 128], DT, tag="w2", name=f"w2_{m}") for m in range(HT)]

        # --- DMA issue: two HW DGE queues (sync + scalar), hand-ordered
        def ldx(j, eng):
            eng.dma_start(xt[j][:], x_d.ap()[:, j * 2 * C : (j + 1) * 2 * C])

        def ldg(m, eng):
            eng.dma_start(wg_t[m][:], w13_d.ap()[m])

        def ldu(m, eng):
            eng.dma_start(wu_t[m][:], w13_d.ap()[m + IT])

        # Every dma_start executes ON its engine, gated by HWDGE flow
        # control (4 transfers in flight per queue) — a queued issue can
        # block the engine for as long as the queue backlog. The sync
        # engine has no compute, so it takes a long up-front issue list;
        # the scalar engine's remaining issues are interleaved into the
        # stage-1 loop below so no silu ever sits behind a gated issue.
        # The sync engine runs no compute, so it takes nearly all weight
        # traffic in strict consumption order (gated issues are harmless
        # there); the scalar engine issues only the startup-critical set
        # -- all its issues clear by ~20us, so silus are never blocked.
        ldg(0, nc.sync)
        ldx(0, nc.sync)
        ldx(1, nc.sync)
        ldg(1, nc.sync)
        ldx(2, nc.sync)
        ldx(3, nc.sync)
        ldu(2, nc.sync)
        for m in range(3, IT):
            ldg(m, nc.sync)
            ldu(m, nc.sync)
        for m in range(HT):
            nc.sync.dma_start(w2_t[m][:], w2_d.ap()[m])
        # scalar queue: startup-critical only (7 issues, all unblocked)
        ldu(0, nc.scalar)
        for j in (4, 5, 6, 7):
            ldx(j, nc.scalar)
        ldu(1, nc.scalar)
        ldg(2, nc.scalar)
        _scalar_late = []

        # x pair-tile j holds k-tiles 2j (cols [0:C]) and 2j+1 (cols [C:2C]);
        # pairs 0-3 arrive via sync, 4-7 via scalar.
        def xs(k):
            return xt[k // 2][:, (k % 2) * C : (k % 2 + 1) * C]

        # --- stage 1+2: guT blocks -> act tiles ------------------------
        # k-consumption order matches x pair arrival (pairs alternate
        # between the two queues); accumulation order is irrelevant
        K_ORDER = [0, 1, 8, 9, 2, 3, 10, 11, 12, 13, 4, 5, 14, 15, 6, 7]
        act_t = []
        for m in range(IT):
            g_ps = psg.tile([128, C], F32, tag="ps")
            u_ps = psg.tile([128, C], F32, tag="ps")
            for i, k in enumerate(K_ORDER):
                nc.tensor.matmul(
                    g_ps[:], wg_t[m][:, k * 128 : (k + 1) * 128], xs(k),
                    start=(i == 0), stop=(i == HT - 1),
                )
            for i, k in enumerate(K_ORDER):
                nc.tensor.matmul(
                    u_ps[:], wu_t[m][:, k * 128 : (k + 1) * 128], xs(k),
                    start=(i == 0), stop=(i == HT - 1),
                )
            sg = sp.tile([128, C], F32, tag="sg")
            nc.scalar.activation(sg[:], g_ps[:], AF.Silu)
            at = ap_.tile([128, C], DT, tag=f"act{m}")
            nc.vector.tensor_mul(at[:], sg[:], u_ps[:])
            act_t.append(at)
            if m < len(_scalar_late):
                for kind, i in _scalar_late[m]:
                    if kind == "g":
                        ldg(i, nc.scalar)
                    elif kind == "u":
                        ldu(i, nc.scalar)
                    else:
                        nc.scalar.dma_start(w2_t[i][:], w2_d.ap()[i])

        # --- stage 3: yT row-blocks ------------------------------------
        # last block runs as two column halves so only a half-width copy
        # + writeback is exposed after the final matmul
        for m in range(HT):
            halves = ((0, C),) if m < HT - 1 else ((0, C // 2), (C // 2, C))
            for c0, c1 in halves:
                y_ps = psy.tile([128, c1 - c0], F32, tag="y")
                for k in range(IT):
                    nc.tensor.matmul(
                        y_ps[:], w2_t[m][:, k * 128 : (k + 1) * 128],
                        act_t[k][:, c0:c1],
                        start=(k == 0), stop=(k == IT - 1),
                    )
                y_sb = sp.tile([128, c1 - c0], DT, tag="yout")
                nc.scalar.copy(y_sb[:], y_ps[:])
                # issue the writeback from the scalar engine right after its
                # copy (no cross-engine semaphore before the DMA can start)
                nc.scalar.dma_start(y_d.ap()[m][:, c0:c1], y_sb[:])

    nc.compile()
    return nc


def _get_nc():
    if "nc" not in _cache:
        _cache["nc"] = _build_nc()
    return _cache["nc"]


def _prep_weights(w13, w2):
    """Pre-tile weights into the SBUF layout the kernel DMAs verbatim.

    w13_sb[e, b, p, k*128+c] = w13[e, b*128+c, k*128+p]   (b: guT row-block)
    w2_sb [e, m, p, k*128+c] = w2 [e, m*128+c, k*128+p]   (m: yT row-block)
    """
    w13_sb = (
        w13.reshape(E, BT, 128, HT, 128)
        .transpose(0, 1, 4, 3, 2)
        .astype(NP_DT)
        .reshape(E, BT, 128, HT * 128)
    )
    w2_sb = (
        w2.reshape(E, HT, 128, IT, 128)
        .transpose(0, 1, 4, 3, 2)
        .astype(NP_DT)
        .reshape(E, HT, 128, IT * 128)
    )
    return w13_sb, w2_sb


def _silu(x):
    return x / (1.0 + np.exp(-x))


def kernel(
    hidden_states,
    topk_weights,
    topk_ids,
    w13,
    w2,
    num_global_tokens=None,
    max_num_tokens_per_gpu=None,
):
    from concourse.bass_utils import run_bass_kernel_spmd

    hs = np.asarray(hidden_states, dtype=np.float32)
    tw = np.asarray(topk_weights, dtype=np.float32)
    ti = np.asarray(topk_ids)
    w13 = np.asarray(w13, dtype=np.float32)
    w2 = np.asarray(w2, dtype=np.float32)

    assert hs.shape == (T, H), hs.shape
    assert w13.shape == (E, 2 * I, H), w13.shape
    assert w2.shape == (E, H, I), w2.shape

    # per-(token, expert) combine weights: sum of topk weights routed to e
    # (out-of-range ids contribute nothing, matching jax.nn.one_hot)
    comb = np.zeros((T, E), dtype=np.float32)
    for k in range(ti.shape[1]):
        col = ti[:, k]
        ok = (col >= 0) & (col < E)
        np.add.at(comb, (np.arange(T)[ok], col[ok]), tw[ok, k])

    idxs = [np.nonzero(comb[:, e])[0] for e in range(E)]

    w13_sb, w2_sb = _prep_weights(w13, w2)
    nc = _get_nc()

    in_maps = []
    sels = []
    for e in range(E):
        sel = idxs[e][:C]
        xe = np.zeros((H, C), dtype=NP_DT)
        xe[:, : len(sel)] = hs[sel].T
        x_sb = np.ascontiguousarray(
            xe.reshape(HT, 128, C).transpose(1, 0, 2)
        ).reshape(128, HT * C)
        in_maps.append({"x_sb": x_sb, "w13_sb": w13_sb[e], "w2_sb": w2_sb[e]})
        sels.append(sel)

    trace = bool(os.environ.get("KERNEL_PROFILE"))
    if trace:
        try:
            res = run_bass_kernel_spmd(nc, in_maps, list(range(E)), trace=True)
            if res.exec_time_ns is not None:
                print(f"HW exec time: {res.exec_time_ns} ns")
        except Exception:
            res = run_bass_kernel_spmd(nc, in_maps, list(range(E)))
    else:
        res = run_bass_kernel_spmd(nc, in_maps, list(range(E)))

    out = np.zeros((T, H), dtype=np.float32)
    for e in range(E):
        sel = sels[e]
        if len(sel):
            y_sb = np.asarray(res.results[e]["y_sb"], dtype=np.float32)
            ye = y_sb.reshape(H, C).T  # [C, H]
            out[sel] += comb[sel, e][:, None] * ye[: len(sel)]
        # overflow beyond the device capacity: exact host compute (fp32)
        of = idxs[e][C:]
        if len(of):
            x = hs[of]
            gu = x @ w13[e].T
            act = _silu(gu[:, :I]) * gu[:, I:]
            y = act @ w2[e].T
            out[of] += comb[of, e][:, None] * y
    return out


# revision 24
# speedup vs baseline: 1.1620x; 1.1620x over previous
"""Trainium2 Bass kernel for a top-2 MoE layer (T=2048, H=2048, I=1408, E=8).

Strategy: expert-parallel over 8 NeuronCores. The host dispatches tokens:
for each expert e it gathers up to C=512 of the tokens routed to e; the
handful of overflow (token, expert) pairs beyond 512 on the busiest experts
(66 pairs for the seed-0 routing) are computed exactly on the host in fp32,
so the device does exactly ONE SPMD launch with a balanced 512-column
capacity on every core.

Device kernel (per core), transposed layout so no on-device transposes:
  stage 1: guT[2816, C] = w13 @ xT         (22 x 16 matmuls, K-tiles of 128)
  stage 2: actT[1408, C] = silu(gT) * uT   (ScalarE Silu + VectorE mul)
  stage 3: yT[2048, C] = w2 @ actT         (16 x 11 matmuls)
Matmuls run in fp16 (full PE rate: 78.6 TF/s/core; fp8 DoubleRow would be
2x but its 3-bit mantissa costs ~4e-2 rel err vs the 2e-2 budget).

Perf notes vs the naive schedule (single qSyncDynamicHW queue ~170 GB/s):
  - DMAs are split across BOTH hardware DGE queues (sync + scalar
    engines) to approach the ~358 GB/s per-core HBM ceiling, with the
    issue order hand-arranged so stage-1 never starves after startup.
  - All w2 tiles are prefetched during stage 1 (they used to arrive
    ~10us late, stalling the tail of stage 3).
  - A burst of dummy warmup matmuls runs during the initial x/w DMA
    window so the PE clock is fully ramped (2.4 GHz p-state) when the
    first real matmul issues.
  - y is written back in fp16 (half the writeback bytes; adds ~2e-4
    rel err against a 2e-2 budget).
"""

import sys

if "/opt/trn_rl_repo" not in sys.path:
    sys.path.insert(0, "/opt/trn_rl_repo")

import os
import numpy as np
from contextlib import ExitStack

import concourse.bass as bass
import concourse.tile as tile
from concourse import bacc, mybir

T, H, I, E, K = 2048, 2048, 1408, 8, 2
C = 512                      # token capacity per expert (one PSUM bank)
HT = H // 128                # 16 K-tiles over H
IT = I // 128                # 11 K-tiles over I
BT = 2 * I // 128            # 22 row-blocks of guT

DT = mybir.dt.float16
NP_DT = np.float16

_cache: dict = {}


def _build_nc():
    """Build + compile the per-core FFN program (same program on all cores)."""
    nc = bacc.Bacc("TRN2", target_bir_lowering=False, debug=False, num_devices=E)
    # x packed partition-major: x_d[p, k*C + c] = x[token c, h = k*128 + p]
    x_d = nc.dram_tensor("x_sb", [128, HT * C], DT, kind="ExternalInput")
    w13_d = nc.dram_tensor("w13_sb", [BT, 128, HT * 128], DT, kind="ExternalInput")
    w2_d = nc.dram_tensor("w2_sb", [HT, 128, IT * 128], DT, kind="ExternalInput")
    y_d = nc.dram_tensor("y_sb", [HT, 128, C], DT, kind="ExternalOutput")

    AF = mybir.ActivationFunctionType
    F32 = mybir.dt.float32

    with tile.TileContext(nc) as tc, ExitStack() as ctx:
        # every weight tile gets its own buffer: load DMA issues then have no
        # WAR deps, so both DGE queues fill their full backlog at t=0 and the
        # issuing engines never block mid-stream (a blocked dma_start on the
        # scalar engine deadlocks against the silu -> PSUM-ring WAR chain)
        xp = ctx.enter_context(tc.tile_pool(name="x", bufs=1))
        wpg = ctx.enter_context(tc.tile_pool(name="wg", bufs=IT))
        wpu = ctx.enter_context(tc.tile_pool(name="wu", bufs=IT))
        w2p = ctx.enter_context(tc.tile_pool(name="w2", bufs=16))
        ap_ = ctx.enter_context(tc.tile_pool(name="act", bufs=1))
        sp = ctx.enter_context(tc.tile_pool(name="tmp", bufs=2))
        psg = ctx.enter_context(
            tc.tile_pool(name="psg", bufs=4, space=bass.MemorySpace.PSUM)
        )
        psy = ctx.enter_context(
            tc.tile_pool(name="psy", bufs=4, space=bass.MemorySpace.PSUM)
        )

        # (no PE warmup: N=128 dummy matmuls run at ~50% utilization — the
        # 128-cycle stationary load isn't hidden — so the clock never ramps
        # and they only delay the first real matmul; the early real matmuls
        # are DMA-paced anyway, which hides the p-state ramp)

        # --- tiles -----------------------------------------------------
        # x as 8 pair-tiles [128, 2C] (2 KB contiguous per partition per DMA)
        xt = [xp.tile([128, 2 * C], DT, tag=f"xp{j}", name=f"xp{j}") for j in range(HT // 2)]
        wg_t = [wpg.tile([128, HT * 128], DT, tag="wg", name=f"wg{m}") for m in range(IT)]
        wu_t = [wpu.tile([128, HT * 128], DT, tag="wu", name=f"wu{m}") for m in range(IT)]
        w2_t = [w2p.tile([128, IT * 128], DT, tag="w2", name=f"w2_{m}") for m in range(HT)]

        # --- DMA issue: two HW DGE queues (sync + scalar), hand-ordered
        def ldx(j, eng):
            eng.dma_start(xt[j][:], x_d.ap()[:, j * 2 * C : (j + 1) * 2 * C])

        def ldg(m, eng):
            eng.dma_start(wg_t[m][:], w13_d.ap()[m])

        def ldu(m, eng):
            eng.dma_start(wu_t[m][:], w13_d.ap()[m + IT])

        # Every dma_start executes ON its engine, gated by HWDGE flow
        # control (4 transfers in flight per queue) — a queued issue can
        # block the engine for as long as the queue backlog. The sync
        # engine has no compute, so it takes a long up-front issue list;
        # the scalar engine's remaining issues are interleaved into the
        # stage-1 loop below so no silu ever sits behind a gated issue.
        # Queues balanced ~9.4 MB each, ordered by consumption deadline.
        # sync (engine has no compute, so long gated issue chains are
        # fine): wg0, x pairs 0-3, wg1..wg10, w2 evens.
        ldg(0, nc.sync)
        for j in (0, 1, 2, 3):
            ldx(j, nc.sync)
        for m in range(1, IT):
            ldg(m, nc.sync)
        for m in range(0, HT, 2):
            nc.sync.dma_start(w2_t[m][:], w2_d.ap()[m])
        # scalar upfront: wu0, x pairs 4-7, wu1, wu2 (gates all clear by
        # ~17us, well before silu m=0)
        ldu(0, nc.scalar)
        for j in (4, 5, 6, 7):
            ldx(j, nc.scalar)
        ldu(1, nc.scalar)
        ldu(2, nc.scalar)
        # one deferred wu per silu keeps the scalar engine's issue backlog
        # at zero when each silu becomes ready
        _scalar_late = [[("u", m)] for m in range(3, IT)]

        # x pair-tile j holds k-tiles 2j (cols [0:C]) and 2j+1 (cols [C:2C]);
        # pairs 0-3 arrive via sync, 4-7 via scalar.
        def xs(k):
            return xt[k // 2][:, (k % 2) * C : (k % 2 + 1) * C]

        # --- stage 1+2: guT blocks -> act tiles ------------------------
        # k-consumption order matches x pair arrival (pairs alternate
        # between the two queues); accumulation order is irrelevant
        K_ORDER = [0, 1, 8, 9, 2, 3, 10, 11, 4, 5, 12, 13, 6, 7, 14, 15]
        act_t = []
        for m in range(IT):
            g_ps = psg.tile([128, C], F32, tag="ps")
            u_ps = psg.tile([128, C], F32, tag="ps")
            for i, k in enumerate(K_ORDER):
                nc.tensor.matmul(
                    g_ps[:], wg_t[m][:, k * 128 : (k + 1) * 128], xs(k),
                    start=(i == 0), stop=(i == HT - 1),
                )
            for i, k in enumerate(K_ORDER):
                nc.tensor.matmul(
                    u_ps[:], wu_t[m][:, k * 128 : (k + 1) * 128], xs(k),
                    start=(i == 0), stop=(i == HT - 1),
                )
            sg = sp.tile([128, C], F32, tag="sg")
            nc.scalar.activation(sg[:], g_ps[:], AF.Silu)
            at = ap_.tile([128, C], DT, tag=f"act{m}")
            nc.vector.tensor_mul(at[:], sg[:], u_ps[:])
            act_t.append(at)
            if m < len(_scalar_late):
                for kind, i in _scalar_late[m]:
                    if kind == "g":
                        ldg(i, nc.scalar)
                    elif kind == "u":
                        ldu(i, nc.scalar)
                    else:
                        nc.scalar.dma_start(w2_t[i][:], w2_d.ap()[i])

        # w2 odds on the scalar queue, issued after the last silu (queue is
        # empty by now so all 8 issues execute immediately; transfers land
        # ~20us before stage 3 needs them)
        for m in range(1, HT, 2):
            nc.scalar.dma_start(w2_t[m][:], w2_d.ap()[m])

        # --- stage 3: yT row-blocks ------------------------------------
        # last block runs as two column halves so only a half-width copy
        # + writeback is exposed after the final matmul
        for m in range(HT):
            halves = ((0, C),) if m < HT - 1 else ((0, C // 2), (C // 2, C))
            for c0, c1 in halves:
                y_ps = psy.tile([128, c1 - c0], F32, tag="y")
                for k in range(IT):
                    nc.tensor.matmul(
                        y_ps[:], w2_t[m][:, k * 128 : (k + 1) * 128],
                        act_t[k][:, c0:c1],
                        start=(k == 0), stop=(k == IT - 1),
                    )
                y_sb = sp.tile([128, c1 - c0], DT, tag="yout")
                nc.scalar.copy(y_sb[:], y_ps[:])
                # issue the writeback from the scalar engine right after its
                # copy (no cross-engine semaphore before the DMA can start)
                nc.scalar.dma_start(y_d.ap()[m][:, c0:c1], y_sb[:])

    nc.compile()
    return nc


def _get_nc():
    if "nc" not in _cache:
        _cache["nc"] = _build_nc()
    return _cache["nc"]


def _prep_weights(w13, w2):
    """Pre-tile weights into the SBUF layout the kernel DMAs verbatim.

    w13_sb[e, b, p, k*128+c] = w13[e, b*128+c, k*128+p]   (b: guT row-block)
    w2_sb [e, m, p, k*128+c] = w2 [e, m*128+c, k*128+p]   (m: yT row-block)
    """
    w13_sb = (
        w13.reshape(E, BT, 128, HT, 128)
        .transpose(0, 1, 4, 3, 2)
        .astype(NP_DT)
        .reshape(E, BT, 128, HT * 128)
    )
    w2_sb = (
        w2.reshape(E, HT, 128, IT, 128)
        .transpose(0, 1, 4, 3, 2)
        .astype(NP_DT)
        .reshape(E, HT, 128, IT * 128)
    )
    return w13_sb, w2_sb


def _silu(x):
    return x / (1.0 + np.exp(-x))


def kernel(
    hidden_states,
    topk_weights,
    topk_ids,
    w13,
    w2,
    num_global_tokens=None,
    max_num_tokens_per_gpu=None,
):
    from concourse.bass_utils import run_bass_kernel_spmd

    hs = np.asarray(hidden_states, dtype=np.float32)
    tw = np.asarray(topk_weights, dtype=np.float32)
    ti = np.asarray(topk_ids)
    w13 = np.asarray(w13, dtype=np.float32)
    w2 = np.asarray(w2, dtype=np.float32)

    assert hs.shape == (T, H), hs.shape
    assert w13.shape == (E, 2 * I, H), w13.shape
    assert w2.shape == (E, H, I), w2.shape

    # per-(token, expert) combine weights: sum of topk weights routed to e
    # (out-of-range ids contribute nothing, matching jax.nn.one_hot)
    comb = np.zeros((T, E), dtype=np.float32)
    for k in range(ti.shape[1]):
        col = ti[:, k]
        ok = (col >= 0) & (col < E)
        np.add.at(comb, (np.arange(T)[ok], col[ok]), tw[ok, k])

    idxs = [np.nonzero(comb[:, e])[0] for e in range(E)]

    w13_sb, w2_sb = _prep_weights(w13, w2)
    nc = _get_nc()

    in_maps = []
    sels = []
    for e in range(E):
        sel = idxs[e][:C]
        xe = np.zeros((H, C), dtype=NP_DT)
        xe[:, : len(sel)] = hs[sel].T
        x_sb = np.ascontiguousarray(
            xe.reshape(HT, 128, C).transpose(1, 0, 2)
        ).reshape(128, HT * C)
        in_maps.append({"x_sb": x_sb, "w13_sb": w13_sb[e], "w2_sb": w2_sb[e]})
        sels.append(sel)

    trace = bool(os.environ.get("KERNEL_PROFILE"))
    if trace:
        try:
            res = run_bass_kernel_spmd(nc, in_maps, list(range(E)), trace=True)
            if res.exec_time_ns is not None:
                print(f"HW exec time: {res.exec_time_ns} ns")
        except Exception:
            res = run_bass_kernel_spmd(nc, in_maps, list(range(E)))
    else:
        res = run_bass_kernel_spmd(nc, in_maps, list(range(E)))

    out = np.zeros((T, H), dtype=np.float32)
    for e in range(E):
        sel = sels[e]
        if len(sel):
            y_sb = np.asarray(res.results[e]["y_sb"], dtype=np.float32)
            ye = y_sb.reshape(H, C).T  # [C, H]
            out[sel] += comb[sel, e][:, None] * ye[: len(sel)]
        # overflow beyond the device capacity: exact host compute (fp32)
        of = idxs[e][C:]
        if len(of):
            x = hs[of]
            gu = x @ w13[e].T
            act = _silu(gu[:, :I]) * gu[:, I:]
            y = act @ w2[e].T
            out[of] += comb[of, e][:, None] * y
    return out
